# revision 17
# baseline (speedup 1.0000x reference)
"""DKEF kernel for Trainium2 (8 NeuronCores, SPMD data-parallel over rows of x).

Math (reference):
  fx = phi_k(x), fz = phi_k(z)            # 3-layer MLP per kernel k (K=3, H=64)
  sq[k,n,m] = ||fx[k,n] - fz[k,m]||^2
  out[n,m]  = sum_k softmax(kernel_weights)[k] * exp(-sq[k,n,m] / (2*10^log_sigma[k]))

v2 design (per core, 2048 rows of x), derived from the v1 trace
(ACT-bound Gram at ~6us/tile, PE stuck at 1.2GHz = 427ns/512-col f32r MM,
front phase 120us with ACT idle ~80us):
  - Native single-pass Softplus (40-ULP table) replaces Exp+Ln per MLP layer.
  - AUG = 65: TX[k] = [fx; 1], BZ[k] = [-2fz; nz]. The x-norm term -c_k*nx
    + ln w_k rides the Gram Exp's per-partition bias column (XBIAS), so no
    nx rows, no hi/lo splits, no [1,n] DVE subtracts, no unaligned DMAs.
    (f32r stores full f32; the PE rounds reads to 13 mantissa bits. Host sim
    of the full rounding chain gives max rel err ~1e-3 for this scheme.)
  - nz written into BZ row 64 by a partition-base-64 DVE copy (32-aligned).
  - Norm squares on ACT (Square is in every table set; ACT has slack vs PE);
    z-side Square uses scale=0.5 so (0.5*(-2fz))^2 = fz^2 and ones weight 1.
  - x-norm bias columns: per row-tile i, 3 tiny matmuls lhsT=sqx[k][:,i*128:+128],
    rhs=(-c_k ones [64,1]) -> psum [128,1]; one DVE add of ln w_k -> XBIAS.
  - Front order: warmup bf16 MM burst (HAM probe, fills the staging dead time)
    -> weights (scalar queue) -> z stage/transpose/MLP -> x ... -> norms -> Gram.
  - Gram: k-outer / h-inner so each TX[k] column-block's weights serve 8 MMs;
    Exp scale = r13(-c_k) matches the r13 ones-weights; bias = XBIAS column.
  - k-sum: 2 bf16 DVE adds (2x mode); bf16 out DMAs on the sync queue, last
    row-tile split into halves to shrink the drain tail.
"""

import sys

for _p in ("/opt/trn_rl_repo",):
    if _p not in sys.path:
        sys.path.insert(0, _p)

from contextlib import ExitStack

import numpy as np

import concourse.bass as bass
import concourse.tile as tile
from concourse import mybir
from concourse.bass_utils import run_bass_kernel_spmd
from concourse.masks import make_identity

K, N, M, D, H = 3, 16384, 4096, 128, 64
N_CORES = 8
NROWS = N // N_CORES  # 2048 rows of x per core

F32 = mybir.dt.float32
F32R = mybir.dt.float32r
BF16 = mybir.dt.bfloat16

P = 128          # partitions
MMF = 512        # max matmul moving free dim (one PSUM bank of fp32)
CH = 1024        # MLP chunk (tokens)
HM = 2048        # Gram m-chunk (4 PSUM banks; 2 chunks double-buffered)
AUG = H + 2      # 66 rows: [fx; 1; nx] x [-2fz; nz; 1]

MDT = F32R

USE_SOFTPLUS = False  # real act_info.json has no softplus table (evicted for act1/act2)


def _wait_limit(inst):
    return 1


def _split_overfull_waits(nc):
    """walrus codegen caps sem waits per instruction (1 for drains and
    fused-ldweights matmuls). Tile can attach more. Peel surplus waits
    onto single-wait NOPs inserted just before the instruction on the
    same engine."""
    items = sorted(
        (int(n.split("-")[1]), n, i)
        for n, i in nc.inst_map.items()
        if n.startswith("I-") and n.split("-")[1].isdigit()
    )
    over = [
        (n, i)
        for _, n, i in items
        if i.sync_info is not None
        and i.sync_info.on_wait
        and len(i.sync_info.on_wait) > _wait_limit(i)
    ]
    if not over:
        return
    blocks = list(nc.m.functions[0].blocks)
    for n, inst in over:
        lim = _wait_limit(inst)
        si = inst.sync_info
        waits = list(si.on_wait)
        keep, surplus = waits[:lim], waits[lim:]
        si.on_wait = keep
        inst.sync_info = si
        eng = nc.engines[inst.engine]
        new_names = []
        for w in surplus:
            nop = eng.nop(hint="wait_split", nofuse=True)
            nsi = nop.ins.sync_info
            if nsi is None:
                nsi = type(si)(on_wait=[w], on_update=[])
            else:
                nsi.on_wait = [w]
            nop.ins.sync_info = nsi
            new_names.append(nop.ins.name)
        moved = False
        for blk in blocks:
            insts = list(blk.instructions)
            names = [x.name for x in insts]
            if n in names:
                all_names = set(names) | {
                    x.name for b in blocks for x in b.instructions
                }
                assert set(new_names) <= all_names
                for b in blocks:
                    bi = list(b.instructions)
                    if any(x.name in new_names for x in bi):
                        b.instructions = [x for x in bi if x.name not in new_names]
                insts = list(blk.instructions)
                keep_objs = [x for x in insts if x.name not in new_names]
                new_objs = [nc.inst_map[m_] for m_ in new_names]
                at = [x.name for x in keep_objs].index(n)
                keep_objs[at:at] = new_objs
                blk.instructions = keep_objs
                moved = True
                break
        assert moved, f"could not find block containing {n}"


def _r13(v):
    """Round a python float to 13 mantissa bits (f32r-exact)."""
    import math
    if v == 0:
        return 0.0
    m, e = math.frexp(v)
    return float(np.float32(math.ldexp(round(m * 8192.0) / 8192.0, e)))


def build_program(n_rows, m, cks, lws, hm=HM):
    """Per-core Bass program. cks = 1/(2*10^log_sigma), lws = ln softmax(kw)."""
    hm = min(hm, m)
    assert n_rows % P == 0 and m % MMF == 0 and hm % MMF == 0 and m % hm == 0
    assert n_rows % CH == 0 and m % CH == 0

    ntiles = n_rows // P
    rc = [_r13(-float(c)) for c in cks]  # r13(-c_k), used both as ones weight
    #                                     and as the Gram Exp scale

    nc = bass.Bass()
    x = nc.declare_dram_parameter("x", [n_rows, D], F32, isOutput=False)
    z = nc.declare_dram_parameter("z", [m, D], F32, isOutput=False)
    W1 = nc.declare_dram_parameter("W1", [K, H, D], F32, isOutput=False)
    b1 = nc.declare_dram_parameter("b1", [K, H], F32, isOutput=False)
    W2 = nc.declare_dram_parameter("W2", [K, H, H], F32, isOutput=False)
    b2 = nc.declare_dram_parameter("b2", [K, H], F32, isOutput=False)
    W3 = nc.declare_dram_parameter("W3", [K, H, H], F32, isOutput=False)
    b3 = nc.declare_dram_parameter("b3", [K, H], F32, isOutput=False)  # unused (cancels)
    out = nc.declare_dram_parameter("out", [n_rows, m], BF16, isOutput=True)

    AF = mybir.ActivationFunctionType
    OP = mybir.AluOpType

    def msetr(ap, v):
        nc.vector.memset(ap.bitcast(F32), _r13(v))

    with ExitStack() as ctx:
        tc = ctx.enter_context(tile.TileContext(nc))
        consts = ctx.enter_context(tc.tile_pool(name="consts", bufs=1))
        big = ctx.enter_context(tc.tile_pool(name="big", bufs=1))

        ident = consts.tile([P, P], F32)
        make_identity(nc, ident)

        # MLP stationaries: k0|k1 stacked; k2 solo.
        SW1p = consts.tile([P, P], MDT, name="SW1p")
        SW1s = consts.tile([P, H], MDT, name="SW1s")
        SW2p = consts.tile([P, P], MDT, name="SW2p")
        SW2s = consts.tile([H, H], MDT, name="SW2s")
        SW3px = consts.tile([P, P], MDT, name="SW3px")
        SW3pz = consts.tile([P, P], MDT, name="SW3pz")
        SW3sx = consts.tile([H, H], MDT, name="SW3sx")
        SW3sz = consts.tile([H, H], MDT, name="SW3sz")
        B1p = consts.tile([P, 1], F32, name="B1p")
        B2p = consts.tile([P, 1], F32, name="B2p")
        B1s = consts.tile([H, 1], F32, name="B1s")
        B2s = consts.tile([H, 1], F32, name="B2s")
        # Norm-matmul weights (z Square runs with scale=0.5, so plain ones
        # suffice on both sides; the Gram Exp scale=-c_k applies to the whole
        # psum including the nx/nz rows).
        onesz = consts.tile([H, 1], MDT, name="onesz")
        msetr(onesz, 1.0)
        # ln w_k columns for the Gram Exp bias.
        BLW = [consts.tile([P, 1], F32, name=f"BLW_{k}") for k in range(K)]
        for k in range(K):
            nc.vector.memset(BLW[k], float(lws[k]))

        # Persistent Gram operands.
        TX = [big.tile([AUG, n_rows], MDT, name=f"TX_{k}") for k in range(K)]
        BZ = [big.tile([AUG, m], MDT, name=f"BZ_{k}") for k in range(K)]
        ones_row = consts.tile([1, CH], F32, name="ones_row")
        nc.gpsimd.memset(ones_row, _r13(1.0))
        for k in range(K):
            nc.gpsimd.memset(TX[k][H : H + 1, :].bitcast(F32), _r13(1.0))

        mid = ctx.enter_context(tc.tile_pool(name="mid", bufs=1))
        xT_h = [mid.tile([P, n_rows], MDT, name="xT"),
                mid.tile([P, m], MDT, name="zT")]

        # ---------------- weights: DMA (scalar queue) + PE transposes ----------
        msetr(SW2p, 0.0)
        msetr(SW3px, 0.0)
        msetr(SW3pz, 0.0)
        with ExitStack() as fctx:
            tp = fctx.enter_context(tc.tile_pool(name="wtp", bufs=3))
            pps = fctx.enter_context(tc.tile_pool(name="wps", bufs=6, space="PSUM"))
            wt = {}
            for k in range(K):
                t = tp.tile([H, D], F32, tag="w1_in")
                nc.scalar.dma_start(out=t, in_=W1[k])
                wt["W1", k] = t
            for k in range(K):
                for nmW, Wsrc in (("W2", W2), ("W3", W3)):
                    t2 = tp.tile([H, H], F32, tag=f"{nmW}_in")
                    nc.scalar.dma_start(out=t2, in_=Wsrc[k])
                    wt[nmW, k] = t2
                for nmB, bsrc in (("b1", b1), ("b2", b2)):
                    row = tp.tile([1, H], F32, tag=f"{nmB}_in")
                    nc.scalar.dma_start(out=row, in_=bsrc[k][None, :])
                    wt[nmB, k] = row
            for k in range(K):
                ps = pps.tile([P, H], F32, tag="ps_t")
                nc.tensor.transpose(ps, wt["W1", k], ident[:H, :H])
                if k < 2:
                    nc.vector.tensor_copy(SW1p[:, k * H : (k + 1) * H], ps)
                else:
                    nc.vector.tensor_copy(SW1s, ps)

                ps2 = pps.tile([H, H], F32, tag="ps_t")
                nc.tensor.transpose(ps2, wt["W2", k], ident[:H, :H])
                if k < 2:
                    nc.vector.tensor_copy(
                        SW2p[k * H : (k + 1) * H, k * H : (k + 1) * H], ps2)
                else:
                    nc.vector.tensor_copy(SW2s, ps2)

                ps3 = pps.tile([H, H], F32, tag="ps_t")
                nc.tensor.transpose(ps3, wt["W3", k], ident[:H, :H])
                if k < 2:
                    sl = slice(k * H, (k + 1) * H)
                    nc.vector.tensor_copy(SW3px[sl, sl], ps3)
                    nc.vector.tensor_scalar(SW3pz[sl, sl], ps3, -2.0, None, OP.mult)
                else:
                    nc.vector.tensor_copy(SW3sx, ps3)
                    nc.vector.tensor_scalar(SW3sz, ps3, -2.0, None, OP.mult)

                for nmB, Bp, Bs in (("b1", B1p, B1s), ("b2", B2p, B2s)):
                    psb = pps.tile([H, 1], F32, tag="ps_t")
                    nc.tensor.transpose(psb, wt[nmB, k], ident[:1, :1])
                    if k < 2:
                        nc.vector.tensor_copy(Bp[k * H : (k + 1) * H, :], psb)
                    else:
                        nc.vector.tensor_copy(Bs, psb)
            # BZ row 65 = 1 (partition 65 is not engine-writable: DMA it).
            # Scalar queue: idle here, keeps sync free for staging.
            for k in range(K):
                for q in range(0, m, CH):
                    nc.scalar.dma_start(
                        out=BZ[k][H + 1 : H + 2, q : q + CH].bitcast(F32),
                        in_=ones_row)

        # ---------------- MLP: stage + transpose + 3 layers, per side ----------
        # Emission order: x side, then z side with the x-norm bias matmuls
        # interleaved into the z chunk loop (they fill PE idle while ACT does
        # z softplus). Packed L3 is deferred one chunk and uP/uS are
        # single-buffered: L1(c+1) can run during Ln(L2,c) because Exp already
        # freed the psum, so ACT never stalls at chunk boundaries.
        def mlp_side(staged, T, tokens, SW3p, SW3s, dsts):
            with ExitStack() as fctx:
                tps = fctx.enter_context(
                    tc.tile_pool(name="tps", bufs=1, space="PSUM"))
                mps = fctx.enter_context(
                    tc.tile_pool(name="mlp_ps", bufs=2, space="PSUM"))
                mpsS = fctx.enter_context(
                    tc.tile_pool(name="mlp_psS", bufs=1, space="PSUM"))
                u3ps = fctx.enter_context(
                    tc.tile_pool(name="u3_ps", bufs=1, space="PSUM", side="right"))
                hp = fctx.enter_context(tc.tile_pool(name="hpool", bufs=2))

                def mm(ps_, lhsT, rhs, parts=P):
                    n_ = rhs.shape[-1]
                    for j in range(0, n_, MMF):
                        nc.tensor.matmul(ps_[0:parts, j : j + MMF], lhsT,
                                         rhs[:, j : j + MMF],
                                         start=True, stop=True)

                def softplus(dst, src, bias):
                    tmp = hp.tile(list(src.shape), MDT, tag="sp_tmp")
                    nc.scalar.activation(tmp, src, AF.Exp, bias=bias)
                    nc.scalar.activation(dst, tmp, AF.Ln, bias=1.0)

                bq = 2

                def stage_chunk(c0):
                    for q0 in range(c0 // P, (c0 + CH) // P, bq):
                        sS = staged[q0]
                        ps = tps.tile([P, bq * P], F32, tag="ps_t2")
                        for i in range(bq):
                            nc.tensor.transpose(
                                ps[:, i * P : (i + 1) * P],
                                sS[:, i * P : (i + 1) * P], ident)
                        nc.vector.tensor_copy(
                            T[:, q0 * P : (q0 + bq) * P], ps)

                def l3_packed(c0):
                    # [128, 512] halves: only 1 spare PSUM bank at this point.
                    for j in range(0, CH, MMF):
                        u3P = u3ps.tile([P, MMF], F32, tag="u3P")
                        nc.tensor.matmul(u3P, SW3p, hps[c0][:, j : j + MMF],
                                         start=True, stop=True)
                        nc.vector.tensor_copy(
                            dsts[0][0:H, c0 + j : c0 + j + MMF], u3P[0:H, :])
                        nc.vector.tensor_copy(
                            dsts[1][0:H, c0 + j : c0 + j + MMF], u3P[H:P, :])

                hps = {}
                for c0 in range(0, tokens, CH):
                    stage_chunk(c0)
                    src = T[:, c0 : c0 + CH]
                    uP = mps.tile([P, CH], F32, tag="uP")
                    mm(uP, SW1p, src)
                    if c0 > 0:
                        l3_packed(c0 - CH)
                    hP = hp.tile([P, CH], MDT, tag="hP")
                    softplus(hP, uP, B1p)
                    uS = mpsS.tile([H, CH], F32, tag="uS")
                    mm(uS, SW1s, src, parts=H)
                    hS = hp.tile([H, CH], MDT, tag="hS")
                    softplus(hS, uS, B1s)

                    u2P = mps.tile([P, CH], F32, tag="uP")
                    mm(u2P, SW2p, hP)
                    h2P = hp.tile([P, CH], MDT, tag="hP")
                    softplus(h2P, u2P, B2p)
                    hps[c0] = h2P
                    u2S = mpsS.tile([H, CH], F32, tag="uS")
                    mm(u2S, SW2s, hS, parts=H)
                    h2S = hp.tile([H, CH], MDT, tag="hS")
                    softplus(h2S, u2S, B2s)

                    # solo L3 inline: reuses the uS buffer (freed by Exp(L2s));
                    # runs on PE during the next chunk's L1 Exp.
                    u3S = mpsS.tile([H, CH], F32, tag="uS")
                    mm(u3S, SW3s, h2S, parts=H)
                    nc.vector.tensor_copy(dsts[2][0:H, c0 : c0 + CH], u3S)
                l3_packed(tokens - CH)

        # All staging DMAs issue up front on the sync queue (x first, then z)
        # so z's 2MB transfer overlaps the x-side MLP entirely.
        stg_scope = ctx.enter_context(ExitStack())
        bq = 2
        staged_maps = {}
        for nm, src_dram, tokens in (("x", x, n_rows), ("z", z, m)):
            pool = stg_scope.enter_context(
                tc.tile_pool(name=f"staging_{nm}", bufs=tokens // P // bq))
            d = {}
            for q0 in range(0, tokens // P, bq):
                sS = pool.tile([P, bq * P], F32, tag="stage")
                nc.sync.dma_start(
                    out=sS[:, :].rearrange("p (b c) -> p b c", c=P),
                    in_=src_dram[q0 * P : (q0 + bq) * P, :].rearrange(
                        "(b p) c -> p b c", p=P),
                )
                d[q0] = sS
            staged_maps[nm] = d

        # x side first: its norms then overlap z staging; z norms (which gate
        # the Gram) come last, emitted half-outer/k-inner so the first Gram
        # tiles can start while later z-norm pieces finish.
        mlp_side(staged_maps["x"], xT_h[0], n_rows, SW3px, SW3sx, TX)

        # x norms: nx -> TX row 65 (partition 65 unaligned: psum -> scratch
        # row -> DMA).
        with ExitStack() as fctx:
            sqp = fctx.enter_context(tc.tile_pool(name="sqpoolx", bufs=2))
            nxp = fctx.enter_context(tc.tile_pool(name="nx_ps", bufs=2, space="PSUM"))
            rp = fctx.enter_context(tc.tile_pool(name="rowsx", bufs=3))
            for k in range(K):
                sq = sqp.tile([H, n_rows], MDT, tag="sqx")
                nc.scalar.activation(sq, TX[k][0:H, :], AF.Square)
                for q in range(0, n_rows, CH):
                    np_ = nxp.tile([1, CH], F32, tag="np")
                    for j in range(0, CH, MMF):
                        nc.tensor.matmul(np_[:, j : j + MMF], onesz,
                                         sq[:, q + j : q + j + MMF],
                                         start=True, stop=True)
                    row = rp.tile([1, CH], MDT, tag="row")
                    nc.vector.tensor_copy(row, np_)
                    nc.sync.dma_start(out=TX[k][H + 1 : H + 2, q : q + CH],
                                      in_=row)

        mlp_side(staged_maps["z"], xT_h[1], m, SW3pz, SW3sz, BZ)
        stg_scope.close()

        # z norms: nz -> BZ row 64 (partition-base-64 DVE copies, 32-aligned).
        with ExitStack() as fctx:
            sqp = fctx.enter_context(tc.tile_pool(name="sqpool", bufs=2))
            nzp = fctx.enter_context(tc.tile_pool(name="nz_ps", bufs=2, space="PSUM"))
            for half in range(0, m, hm):
                for k in range(K):
                    sq = sqp.tile([H, hm], MDT, tag="sqz")
                    nc.scalar.activation(sq, BZ[k][0:H, half : half + hm],
                                         AF.Square, scale=0.5)
                    for q in range(0, hm, CH):
                        npz = nzp.tile([1, CH], F32, tag="npz")
                        for j in range(0, CH, MMF):
                            nc.tensor.matmul(npz[:, j : j + MMF], onesz,
                                             sq[:, q + j : q + j + MMF],
                                             start=True, stop=True)
                        nc.vector.tensor_copy(
                            BZ[k][H : H + 1, half + q : half + q + CH], npz)

        # ---------------- Gram + exp + k-sum ----------------
        with ExitStack() as gctx:
            gps = gctx.enter_context(tc.tile_pool(name="gram_ps", bufs=2, space="PSUM"))
            ep = gctx.enter_context(tc.tile_pool(name="epool", bufs=2))
            op_ = gctx.enter_context(tc.tile_pool(name="opool", bufs=2))

            for i in range(ntiles):
                n0 = i * P
                es = {}
                for k in range(K):
                    for h0 in range(0, m, hm):
                        ps = gps.tile([P, hm], F32, tag="gram")
                        for mt in range(0, hm, MMF):
                            nc.tensor.matmul(
                                ps[:, mt : mt + MMF],
                                TX[k][:, n0 : n0 + P],
                                BZ[k][:, h0 + mt : h0 + mt + MMF],
                                start=True, stop=True,
                            )
                        e = ep.tile([P, hm], BF16, tag=f"e{k}h{h0}")
                        nc.scalar.activation(e, ps, AF.Exp, scale=rc[k],
                                             bias=BLW[k])
                        es[k, h0] = e
                    if k == 1:
                        for h0 in range(0, m, hm):
                            t01 = ep.tile([P, hm], BF16, tag=f"t01h{h0}")
                            nc.vector.tensor_tensor(t01, es[0, h0], es[1, h0],
                                                    OP.add)
                            es["t", h0] = t01
                for h0 in range(0, m, hm):
                    ot = op_.tile([P, hm], BF16, tag=f"ot{h0}")
                    nc.vector.tensor_tensor(ot, es["t", h0], es[2, h0], OP.add)
                    nsplit = 4 if i == ntiles - 1 else 1
                    step = hm // nsplit
                    for s0 in range(0, hm, step):
                        nc.sync.dma_start(
                            out=out[n0 : n0 + P, h0 + s0 : h0 + s0 + step],
                            in_=ot[:, s0 : s0 + step])

    _split_overfull_waits(nc)
    nc.finalize()
    return nc


def _host_prep(inputs):
    ls = np.asarray(inputs["log_sigma"], np.float64)
    kw = np.asarray(inputs["kernel_weights"], np.float64)
    cks = 1.0 / (2.0 * np.power(10.0, ls))
    w = np.exp(kw - kw.max())
    w = w / w.sum()
    lws = np.log(w)
    return cks, lws


def run(inputs, trace=False, n_cores=N_CORES):
    cks, lws = _host_prep(inputs)
    nc = build_program(NROWS, M, cks, lws)
    x = np.ascontiguousarray(np.asarray(inputs["x"], np.float32))
    shared = {
        name: np.ascontiguousarray(np.asarray(inputs[name], np.float32))
        for name in ("z", "W1", "b1", "W2", "b2", "W3", "b3")
    }
    in_maps = [
        {"x": x[c * NROWS : (c + 1) * NROWS], **shared} for c in range(n_cores)
    ]
    res = run_bass_kernel_spmd(nc, in_maps, list(range(n_cores)), trace=trace)
    outs = [np.asarray(res.results[c]["out"]).astype(np.float32)
            for c in range(n_cores)]
    return np.concatenate(outs, axis=0), res


def kernel(**inputs) -> np.ndarray:
    out, _ = run(inputs, trace=False)
    return out


# revision 35
# speedup vs baseline: 1.2352x; 1.2352x over previous
"""DKEF kernel for Trainium2 (8 NeuronCores, SPMD data-parallel over rows of x).

Math (reference):
  fx = phi_k(x), fz = phi_k(z)            # 3-layer MLP per kernel k (K=3, H=64)
  sq[k,n,m] = ||fx[k,n] - fz[k,m]||^2
  out[n,m]  = sum_k softmax(kernel_weights)[k] * exp(-sq[k,n,m] / (2*10^log_sigma[k]))

Design (per core, 2048 rows of x; ~337us at the 1.2GHz device state):
  - AUG=66 Gram operands: TX[k]=[fx; 1; nx], BZ[k]=[-2fz; nz; 1], all f32r
    (full f32 in SBUF; the PE rounds reads to 13 mantissa bits). One f32r
    matmul per (row-tile, k, m-chunk); Exp(scale=r13(-c_k), bias=ln w_k).
  - ACT is the bottleneck (1 elem/cycle/lane): 96 Gram Exps of [128,2048]
    (~2us each) + the MLP softplus chain. No native softplus table in this
    toolchain, so softplus = Exp then Ln(bias=1) (both + Square live in one
    ACT table set -> single table load).
  - MLP runs in chunk-pairs of 1024 tokens: two packed (k0|k1) Exps share one
    merged [128,2048] Ln; packed-L3 is deferred one pair (inline for the last
    pair) so ACT never stalls at pair boundaries; uP double-buffered.
    PSUM per side: transposes 1 bank + uP 4 + uS 2 + deferred-L3 1 = 8.
  - Front: big staging DMAs (x on sync queue, z on scalar; DMA *issue* costs
    ~0.8us each on the queue engine, so few big ones), PE transposes from
    [128,1024] stage tiles, weights batched into 3 rearranged DMAs.
  - Norms: Square on ACT; ones-matmuls reduce to [1,*] psum rows; nz -> BZ
    row 64 via partition-base-64 DVE copy (32-aligned); nx -> scratch row ->
    DMA into TX row 65; BZ row 65 (const 1) DMA'd from a memset scratch row
    (engine writes need 32-aligned partition bases, DMAs don't).
  - z-norms emitted half-outer/k-inner so the first Gram tiles start while
    later pieces finish. k-sum: 2 bf16 DVE adds (2x mode); bf16 out DMAs on
    sync, last row-tile split 4-ways to shrink the drain tail.
  - Known non-fixes (measured): bf16 anywhere in the k2 MLP breaks the 2e-2
    rel-err gate (3.4e-2); bf16 "warmer" matmuls to lift the PE HAM throttle
    correlate with a chip-wide 1.2->1.0GHz downclock; f32r matmuls neither
    feed the HAM nor benefit reliably from it; the device also flips into
    the 1.0GHz state run-to-run on its own (~20% time penalty, not
    kernel-controllable).
The max(sq, 0) clamp in the reference is a no-op for this data distribution
(min sq ~ 2.1) and is omitted. b3 cancels in the pairwise distance.
"""

import sys

for _p in ("/opt/trn_rl_repo",):
    if _p not in sys.path:
        sys.path.insert(0, _p)

from contextlib import ExitStack

import numpy as np

import concourse.bass as bass
import concourse.tile as tile
from concourse import mybir
from concourse.bass_utils import run_bass_kernel_spmd
from concourse.masks import make_identity

K, N, M, D, H = 3, 16384, 4096, 128, 64
N_CORES = 8
NROWS = N // N_CORES  # 2048 rows of x per core

F32 = mybir.dt.float32
F32R = mybir.dt.float32r
BF16 = mybir.dt.bfloat16

P = 128          # partitions
MMF = 512        # max matmul moving free dim (one PSUM bank of fp32)
CH = 1024        # MLP chunk (tokens)
HM = 2048        # Gram m-chunk (4 PSUM banks; 2 chunks double-buffered)
AUG = H + 2      # 66 rows: [fx; 1; nx] x [-2fz; nz; 1]

MDT = F32R

USE_SOFTPLUS = False  # real act_info.json has no softplus table (evicted for act1/act2)


def _wait_limit(inst):
    return 1


def _split_overfull_waits(nc):
    """walrus codegen caps sem waits per instruction (1 for drains and
    fused-ldweights matmuls). Tile can attach more. Peel surplus waits
    onto single-wait NOPs inserted just before the instruction on the
    same engine."""
    items = sorted(
        (int(n.split("-")[1]), n, i)
        for n, i in nc.inst_map.items()
        if n.startswith("I-") and n.split("-")[1].isdigit()
    )
    over = [
        (n, i)
        for _, n, i in items
        if i.sync_info is not None
        and i.sync_info.on_wait
        and len(i.sync_info.on_wait) > _wait_limit(i)
    ]
    if not over:
        return
    blocks = list(nc.m.functions[0].blocks)
    for n, inst in over:
        lim = _wait_limit(inst)
        si = inst.sync_info
        waits = list(si.on_wait)
        keep, surplus = waits[:lim], waits[lim:]
        si.on_wait = keep
        inst.sync_info = si
        eng = nc.engines[inst.engine]
        new_names = []
        for w in surplus:
            nop = eng.nop(hint="wait_split", nofuse=True)
            nsi = nop.ins.sync_info
            if nsi is None:
                nsi = type(si)(on_wait=[w], on_update=[])
            else:
                nsi.on_wait = [w]
            nop.ins.sync_info = nsi
            new_names.append(nop.ins.name)
        moved = False
        for blk in blocks:
            insts = list(blk.instructions)
            names = [x.name for x in insts]
            if n in names:
                all_names = set(names) | {
                    x.name for b in blocks for x in b.instructions
                }
                assert set(new_names) <= all_names
                for b in blocks:
                    bi = list(b.instructions)
                    if any(x.name in new_names for x in bi):
                        b.instructions = [x for x in bi if x.name not in new_names]
                insts = list(blk.instructions)
                keep_objs = [x for x in insts if x.name not in new_names]
                new_objs = [nc.inst_map[m_] for m_ in new_names]
                at = [x.name for x in keep_objs].index(n)
                keep_objs[at:at] = new_objs
                blk.instructions = keep_objs
                moved = True
                break
        assert moved, f"could not find block containing {n}"


def _r13(v):
    """Round a python float to 13 mantissa bits (f32r-exact)."""
    import math
    if v == 0:
        return 0.0
    m, e = math.frexp(v)
    return float(np.float32(math.ldexp(round(m * 8192.0) / 8192.0, e)))


def build_program(n_rows, m, cks, lws, hm=HM):
    """Per-core Bass program. cks = 1/(2*10^log_sigma), lws = ln softmax(kw)."""
    hm = min(hm, m)
    assert n_rows % P == 0 and m % MMF == 0 and hm % MMF == 0 and m % hm == 0
    assert n_rows % CH == 0 and m % CH == 0

    ntiles = n_rows // P
    rc = [_r13(-float(c)) for c in cks]  # r13(-c_k), used both as ones weight
    #                                     and as the Gram Exp scale

    nc = bass.Bass()
    x = nc.declare_dram_parameter("x", [n_rows, D], F32, isOutput=False)
    z = nc.declare_dram_parameter("z", [m, D], F32, isOutput=False)
    W1 = nc.declare_dram_parameter("W1", [K, H, D], F32, isOutput=False)
    b1 = nc.declare_dram_parameter("b1", [K, H], F32, isOutput=False)
    W2 = nc.declare_dram_parameter("W2", [K, H, H], F32, isOutput=False)
    b2 = nc.declare_dram_parameter("b2", [K, H], F32, isOutput=False)
    W3 = nc.declare_dram_parameter("W3", [K, H, H], F32, isOutput=False)
    b3 = nc.declare_dram_parameter("b3", [K, H], F32, isOutput=False)  # unused (cancels)
    out = nc.declare_dram_parameter("out", [n_rows, m], BF16, isOutput=True)

    AF = mybir.ActivationFunctionType
    OP = mybir.AluOpType

    def msetr(ap, v):
        nc.vector.memset(ap.bitcast(F32), _r13(v))

    with ExitStack() as ctx:
        tc = ctx.enter_context(tile.TileContext(nc))
        consts = ctx.enter_context(tc.tile_pool(name="consts", bufs=1))
        big = ctx.enter_context(tc.tile_pool(name="big", bufs=1))

        ident = consts.tile([P, P], F32)
        make_identity(nc, ident)


        # MLP stationaries: k0|k1 stacked; k2 solo.
        SW1p = consts.tile([P, P], MDT, name="SW1p")
        SW1s = consts.tile([P, H], MDT, name="SW1s")
        SW2p = consts.tile([P, P], MDT, name="SW2p")
        SW2s = consts.tile([H, H], MDT, name="SW2s")
        SW3px = consts.tile([P, P], MDT, name="SW3px")
        SW3pz = consts.tile([P, P], MDT, name="SW3pz")
        SW3sx = consts.tile([H, H], MDT, name="SW3sx")
        SW3sz = consts.tile([H, H], MDT, name="SW3sz")
        B1p = consts.tile([P, 1], F32, name="B1p")
        B2p = consts.tile([P, 1], F32, name="B2p")
        B1s = consts.tile([H, 1], F32, name="B1s")
        B2s = consts.tile([H, 1], F32, name="B2s")
        # Norm-matmul weights (z Square runs with scale=0.5, so plain ones
        # suffice on both sides; the Gram Exp scale=-c_k applies to the whole
        # psum including the nx/nz rows).
        onesz = consts.tile([H, 1], MDT, name="onesz")
        msetr(onesz, 1.0)
        # ln w_k columns for the Gram Exp bias.
        BLW = [consts.tile([P, 1], F32, name=f"BLW_{k}") for k in range(K)]
        for k in range(K):
            nc.vector.memset(BLW[k], float(lws[k]))

        # Persistent Gram operands.
        TX = [big.tile([AUG, n_rows], MDT, name=f"TX_{k}") for k in range(K)]
        BZ = [big.tile([AUG, m], MDT, name=f"BZ_{k}") for k in range(K)]
        ones_row = consts.tile([1, hm], F32, name="ones_row")

        mid = ctx.enter_context(tc.tile_pool(name="mid", bufs=1))
        xT_h = [mid.tile([P, n_rows], MDT, name="xT"),
                mid.tile([P, m], MDT, name="zT")]

        # ---------------- weights: DMA (scalar queue) + PE transposes ----------
        msetr(SW2p, 0.0)
        msetr(SW3px, 0.0)
        msetr(SW3pz, 0.0)
        with ExitStack() as fctx:
            tp = fctx.enter_context(tc.tile_pool(name="wtp", bufs=3))
            pps = fctx.enter_context(tc.tile_pool(name="wps", bufs=6, space="PSUM"))
            wt = {}
            w1s = tp.tile([H, K * D], F32, tag="w1_in")
            nc.scalar.dma_start(
                out=w1s.rearrange("h (k d) -> h k d", k=K),
                in_=W1[:, :, :].rearrange("k h d -> h k d"))
            for k in range(K):
                wt["W1", k] = w1s[:, k * D : (k + 1) * D]
            for nmW, Wsrc in (("W2", W2), ("W3", W3)):
                ws = tp.tile([H, K * H], F32, tag=f"{nmW}_in")
                nc.scalar.dma_start(
                    out=ws.rearrange("g (k h) -> g k h", k=K),
                    in_=Wsrc[:, :, :].rearrange("k g h -> g k h"))
                for k in range(K):
                    wt[nmW, k] = ws[:, k * H : (k + 1) * H]
            for k in range(K):
                for nmB, bsrc in (("b1", b1), ("b2", b2)):
                    row = tp.tile([1, H], F32, tag=f"{nmB}_in")
                    nc.scalar.dma_start(out=row, in_=bsrc[k][None, :])
                    wt[nmB, k] = row
            for k in range(K):
                ps = pps.tile([P, H], F32, tag="ps_t")
                nc.tensor.transpose(ps, wt["W1", k], ident[:H, :H])
                if k < 2:
                    nc.vector.tensor_copy(SW1p[:, k * H : (k + 1) * H], ps)
                else:
                    nc.vector.tensor_copy(SW1s, ps)

                ps2 = pps.tile([H, H], F32, tag="ps_t")
                nc.tensor.transpose(ps2, wt["W2", k], ident[:H, :H])
                if k < 2:
                    nc.vector.tensor_copy(
                        SW2p[k * H : (k + 1) * H, k * H : (k + 1) * H], ps2)
                else:
                    nc.vector.tensor_copy(SW2s, ps2)

                ps3 = pps.tile([H, H], F32, tag="ps_t")
                nc.tensor.transpose(ps3, wt["W3", k], ident[:H, :H])
                if k < 2:
                    sl = slice(k * H, (k + 1) * H)
                    nc.vector.tensor_copy(SW3px[sl, sl], ps3)
                    nc.vector.tensor_scalar(SW3pz[sl, sl], ps3, -2.0, None, OP.mult)
                else:
                    nc.vector.tensor_copy(SW3sx, ps3)
                    nc.vector.tensor_scalar(SW3sz, ps3, -2.0, None, OP.mult)

                for nmB, Bp, Bs in (("b1", B1p, B1s), ("b2", B2p, B2s)):
                    psb = pps.tile([H, 1], F32, tag="ps_t")
                    nc.tensor.transpose(psb, wt[nmB, k], ident[:1, :1])
                    if k < 2:
                        nc.vector.tensor_copy(Bp[k * H : (k + 1) * H, :], psb)
                    else:
                        nc.vector.tensor_copy(Bs, psb)

        # ---------------- MLP: stage + transpose + 3 layers, per side ----------
        # Emission order: x side, then z side with the x-norm bias matmuls
        # interleaved into the z chunk loop (they fill PE idle while ACT does
        # z softplus). Packed L3 is deferred one chunk and uP/uS are
        # single-buffered: L1(c+1) can run during Ln(L2,c) because Exp already
        # freed the psum, so ACT never stalls at chunk boundaries.
        def mlp_side(staged, T, tokens, SW3p, SW3s, dsts):
            # Chunk-PAIR pipeline. The k2 "solo" stream packs chunk A on
            # partitions 0-63 and chunk B on 64-127 (col-tiled matmuls), so
            # its Exp/Ln run at [128, CH] — half the ACT calls. Packed-L3 is
            # deferred one pair so ACT never waits at pair boundaries.
            with ExitStack() as fctx:
                tps = fctx.enter_context(
                    tc.tile_pool(name="tps", bufs=1, space="PSUM"))
                mps = fctx.enter_context(
                    tc.tile_pool(name="mlp_ps", bufs=2, space="PSUM"))
                mpsS = fctx.enter_context(
                    tc.tile_pool(name="mlp_psS", bufs=1, space="PSUM"))
                u3ps = fctx.enter_context(
                    tc.tile_pool(name="u3_ps", bufs=1, space="PSUM", side="right"))
                hp = fctx.enter_context(tc.tile_pool(name="hpool", bufs=2))
                hp1 = fctx.enter_context(tc.tile_pool(name="hpool1", bufs=1))

                def mmf(ps_, lhsT, rhs, parts=slice(0, P), tile_position=None):
                    n_ = rhs.shape[-1]
                    for j in range(0, n_, MMF):
                        nc.tensor.matmul(ps_[parts, j : j + MMF], lhsT,
                                         rhs[:, j : j + MMF], start=True,
                                         stop=True, tile_position=tile_position)

                def stage_chunk(c0):
                    sS = staged[c0 // P]
                    for half in range(0, CH, MMF):
                        ps = tps.tile([P, MMF], F32, tag="ps_t2")
                        for i in range(0, MMF, P):
                            nc.tensor.transpose(
                                ps[:, i : i + P],
                                sS[:, half + i : half + i + P], ident)
                        nc.vector.tensor_copy(
                            T[:, c0 + half : c0 + half + MMF], ps)

                def l3_packed(c0, h2P):
                    for j in range(0, CH, MMF):
                        u3P = u3ps.tile([P, MMF], F32, tag="u3P")
                        nc.tensor.matmul(u3P, SW3p, h2P[:, j : j + MMF],
                                         start=True, stop=True)
                        nc.vector.tensor_copy(
                            dsts[0][0:H, c0 + j : c0 + j + MMF], u3P[0:H, :])
                        nc.vector.tensor_copy(
                            dsts[1][0:H, c0 + j : c0 + j + MMF], u3P[H:P, :])

                def packed_layer(Wp, srcA, srcB, bias):
                    # two Exps into one tmp pair, one merged Ln
                    uA = mps.tile([P, CH], F32, tag="uP")
                    mmf(uA, Wp, srcA)
                    tmp = hp1.tile([P, 2 * CH], MDT, tag="tmpP")
                    nc.scalar.activation(tmp[:, :CH], uA, AF.Exp, bias=bias)
                    uB = mps.tile([P, CH], F32, tag="uP")
                    mmf(uB, Wp, srcB)
                    nc.scalar.activation(tmp[:, CH:], uB, AF.Exp, bias=bias)
                    hPp = hp.tile([P, 2 * CH], MDT, tag="hPp")
                    nc.scalar.activation(hPp, tmp, AF.Ln, bias=1.0)
                    return hPp

                def solo_pair(Ws, srcA, srcB, bias):
                    # two Exps into one tmp, one merged [64, 2CH] Ln
                    tmpS = hp1.tile([H, 2 * CH], MDT, tag="tmpS")
                    uSA = mpsS.tile([H, CH], F32, tag="uS")
                    mmf(uSA, Ws, srcA, parts=slice(0, H))
                    nc.scalar.activation(tmpS[:, :CH], uSA, AF.Exp, bias=bias)
                    uSB = mpsS.tile([H, CH], F32, tag="uS")
                    mmf(uSB, Ws, srcB, parts=slice(0, H))
                    nc.scalar.activation(tmpS[:, CH:], uSB, AF.Exp, bias=bias)
                    hSp = hp1.tile([H, 2 * CH], MDT, tag="hSp")
                    nc.scalar.activation(hSp, tmpS, AF.Ln, bias=1.0)
                    return hSp

                prev = None
                last = tokens - 2 * CH
                for c0 in range(0, tokens, 2 * CH):
                    cA, cB = c0, c0 + CH
                    stage_chunk(cA)
                    stage_chunk(cB)
                    srcA = T[:, cA : cA + CH]
                    srcB = T[:, cB : cB + CH]
                    h1P = packed_layer(SW1p, srcA, srcB, B1p)
                    if prev is not None:
                        l3_packed(prev[0], prev[1][:, :CH])
                        l3_packed(prev[0] + CH, prev[1][:, CH:])
                    h1S = solo_pair(SW1s, srcA, srcB, B1s)
                    h2P = packed_layer(SW2p, h1P[:, :CH], h1P[:, CH:], B2p)
                    if c0 == last:
                        # last pair: emit packed L3 inline so its drain overlaps
                        # the solo L2/L3 ACT calls instead of stalling the next
                        # phase's squares.
                        l3_packed(cA, h2P[:, :CH])
                        l3_packed(cB, h2P[:, CH:])
                    h2S = solo_pair(SW2s, h1S[:, :CH], h1S[:, CH:], B2s)
                    for cX, sl in ((cA, slice(0, CH)), (cB, slice(CH, 2 * CH))):
                        u3S = mpsS.tile([H, CH], F32, tag="uS")
                        mmf(u3S, SW3s, h2S[:, sl], parts=slice(0, H))
                        nc.vector.tensor_copy(dsts[2][0:H, cX : cX + CH], u3S)
                    prev = (cA, h2P) if c0 != last else None

        # Staging: few big DMAs (issue cost ~800ns each is the front's real
        # constraint). x on the sync queue, z on scalar (parallel issue);
        # the BZ ones-rows ride sync afterwards.
        stg_scope = ctx.enter_context(ExitStack())
        bq = CH // P  # 8 blocks per stage tile
        staged_maps = {}
        for nm, src_dram, tokens, queue in (
                ("x", x, n_rows, nc.sync), ("z", z, m, nc.scalar)):
            pool = stg_scope.enter_context(
                tc.tile_pool(name=f"staging_{nm}", bufs=tokens // CH))
            d = {}
            for q0 in range(0, tokens // P, bq):
                sS = pool.tile([P, CH], F32, tag="stage")
                nsp = 2 if (nm == "x" and q0 == 0) else 1
                hb = bq // nsp
                for s in range(nsp):
                    qs = q0 + s * hb
                    queue.dma_start(
                        out=sS[:, s * hb * P : (s + 1) * hb * P].rearrange(
                            "p (b c) -> p b c", c=P),
                        in_=src_dram[qs * P : (qs + hb) * P, :].rearrange(
                            "(b p) c -> p b c", p=P),
                    )
                d[q0] = sS
            staged_maps[nm] = d
        # memsets emitted after the staging DMAs so the sync queue's first
        # descriptors don't inherit waits on gpsimd.
        nc.gpsimd.memset(ones_row, _r13(1.0))
        for k in range(K):
            nc.gpsimd.memset(TX[k][H : H + 1, :].bitcast(F32), _r13(1.0))
        for k in range(K):
            for q in range(0, m, hm):
                nc.sync.dma_start(
                    out=BZ[k][H + 1 : H + 2, q : q + hm].bitcast(F32),
                    in_=ones_row)

        # x side first: its norms then overlap z staging; z norms (which gate
        # the Gram) come last, emitted half-outer/k-inner so the first Gram
        # tiles can start while later z-norm pieces finish.
        mlp_side(staged_maps["x"], xT_h[0], n_rows, SW3px, SW3sx, TX)

        # x norms: only the squares run here (pure ACT, no psum) so the z
        # side's transposes/L1 start immediately on the PE; the nx reduction
        # is deferred into the z-norm scope. sqx pool on the right heap side
        # (the left-side staging pools close later: LIFO per side).
        sqx_scope = ctx.enter_context(ExitStack())
        sqxp = sqx_scope.enter_context(
            tc.tile_pool(name="sqxpool", bufs=1, side="right"))
        sqx = []
        for k in range(K):
            sq = sqxp.tile([H, n_rows], MDT, tag=f"sqx{k}")
            nc.scalar.activation(sq, TX[k][0:H, :], AF.Square)
            sqx.append(sq)

        mlp_side(staged_maps["z"], xT_h[1], m, SW3pz, SW3sz, BZ)
        stg_scope.close()

        # Norms: deferred nx first (gates only the Gram), then z nz rows.
        with ExitStack() as fctx:
            sqp = fctx.enter_context(tc.tile_pool(name="sqpool", bufs=2))
            nzp = fctx.enter_context(tc.tile_pool(name="nz_ps", bufs=2, space="PSUM"))
            rp = fctx.enter_context(tc.tile_pool(name="rowsx", bufs=3))
            for k in range(K):
                for q in range(0, n_rows, CH):
                    np_ = nzp.tile([1, CH], F32, tag="npz")
                    for j in range(0, CH, MMF):
                        nc.tensor.matmul(np_[:, j : j + MMF], onesz,
                                         sqx[k][:, q + j : q + j + MMF],
                                         start=True, stop=True)
                    row = rp.tile([1, CH], MDT, tag="row")
                    nc.vector.tensor_copy(row, np_)
                    nc.sync.dma_start(out=TX[k][H + 1 : H + 2, q : q + CH],
                                      in_=row)
            for half in range(0, m, hm):
                for k in range(K):
                    sq = sqp.tile([H, hm], MDT, tag="sqz")
                    nc.scalar.activation(sq, BZ[k][0:H, half : half + hm],
                                         AF.Square, scale=0.5)
                    for q in range(0, hm, CH):
                        npz = nzp.tile([1, CH], F32, tag="npz")
                        for j in range(0, CH, MMF):
                            nc.tensor.matmul(npz[:, j : j + MMF], onesz,
                                             sq[:, q + j : q + j + MMF],
                                             start=True, stop=True)
                        nc.vector.tensor_copy(
                            BZ[k][H : H + 1, half + q : half + q + CH], npz)

        sqx_scope.close()

        # ---------------- Gram + exp + k-sum ----------------
        with ExitStack() as gctx:
            gps = gctx.enter_context(tc.tile_pool(name="gram_ps", bufs=2, space="PSUM"))
            ep = gctx.enter_context(tc.tile_pool(name="epool", bufs=2))
            op_ = gctx.enter_context(tc.tile_pool(name="opool", bufs=2))

            for i in range(ntiles):
                n0 = i * P
                es = {}
                for k in range(K):
                    for h0 in range(0, m, hm):
                        ps = gps.tile([P, hm], F32, tag="gram")
                        for mt in range(0, hm, MMF):
                            nc.tensor.matmul(
                                ps[:, mt : mt + MMF],
                                TX[k][:, n0 : n0 + P],
                                BZ[k][:, h0 + mt : h0 + mt + MMF],
                                start=True, stop=True,
                            )
                        e = ep.tile([P, hm], BF16, tag=f"e{k}h{h0}")
                        nc.scalar.activation(e, ps, AF.Exp, scale=rc[k],
                                             bias=BLW[k])
                        es[k, h0] = e
                    if k == 1:
                        for h0 in range(0, m, hm):
                            t01 = ep.tile([P, hm], BF16, tag=f"t01h{h0}")
                            nc.vector.tensor_tensor(t01, es[0, h0], es[1, h0],
                                                    OP.add)
                            es["t", h0] = t01
                for h0 in range(0, m, hm):
                    ot = op_.tile([P, hm], BF16, tag=f"ot{h0}")
                    if i == ntiles - 1:
                        # halve the final adds so the out DMAs start earlier
                        for s0 in range(0, hm, hm // 2):
                            sl = slice(s0, s0 + hm // 2)
                            nc.vector.tensor_tensor(ot[:, sl], es["t", h0][:, sl],
                                                    es[2, h0][:, sl], OP.add)
                            for d0 in range(s0, s0 + hm // 2, hm // 4):
                                nc.sync.dma_start(
                                    out=out[n0 : n0 + P,
                                            h0 + d0 : h0 + d0 + hm // 4],
                                    in_=ot[:, d0 : d0 + hm // 4])
                        continue
                    nc.vector.tensor_tensor(ot, es["t", h0], es[2, h0], OP.add)
                    nsplit = 2 if i == ntiles - 2 else 1
                    step = hm // nsplit
                    for s0 in range(0, hm, step):
                        nc.sync.dma_start(
                            out=out[n0 : n0 + P, h0 + s0 : h0 + s0 + step],
                            in_=ot[:, s0 : s0 + step])

    _split_overfull_waits(nc)
    nc.finalize()
    return nc


def _host_prep(inputs):
    ls = np.asarray(inputs["log_sigma"], np.float64)
    kw = np.asarray(inputs["kernel_weights"], np.float64)
    cks = 1.0 / (2.0 * np.power(10.0, ls))
    w = np.exp(kw - kw.max())
    w = w / w.sum()
    lws = np.log(w)
    return cks, lws


def run(inputs, trace=False, n_cores=N_CORES):
    cks, lws = _host_prep(inputs)
    nc = build_program(NROWS, M, cks, lws)
    x = np.ascontiguousarray(np.asarray(inputs["x"], np.float32))
    shared = {
        name: np.ascontiguousarray(np.asarray(inputs[name], np.float32))
        for name in ("z", "W1", "b1", "W2", "b2", "W3", "b3")
    }
    in_maps = [
        {"x": x[c * NROWS : (c + 1) * NROWS], **shared} for c in range(n_cores)
    ]
    res = run_bass_kernel_spmd(nc, in_maps, list(range(n_cores)), trace=trace)
    outs = [np.asarray(res.results[c]["out"]).astype(np.float32)
            for c in range(n_cores)]
    return np.concatenate(outs, axis=0), res


def kernel(**inputs) -> np.ndarray:
    out, _ = run(inputs, trace=False)
    return out
